# revision 26
# baseline (speedup 1.0000x reference)
"""Column-L2-normalization kernel for Trainium2 (8 NeuronCores, SPMD).

Computes y = x / sqrt(sum(x*x, axis=0)) for x of shape (524288, 256) fp32.

Strategy (row-sharded data parallel, single kernel launch):
  - Each of the 8 cores gets a contiguous shard of 65536 rows (64 MB).
  - Pass 1: stream 1 MB tiles ([128 partitions x 2048 fp32], 8 rows per
    partition), square on the scalar (ACT) engine, reduce over partitions
    with a ones-vector matmul accumulating into PSUM across all tiles.
    The last NRES tiles stay resident in SBUF (squared into scratch so
    the x data survives) and are not re-read in pass 2.
  - Reduce the row axis of the PSUM accumulator on DVE, AllReduce the
    256 per-column sums across the 8 cores, take 1/sqrt, and broadcast
    across partitions with a rank-1 matmul into PSUM.
  - Pass 2: resident tiles are scaled and stored immediately; the rest
    of the shard is re-streamed, scaled on DVE, and written out.
"""

import numpy as np

import concourse.bacc as bacc
import concourse.mybir as mybir
from concourse import tile
from concourse.bass_utils import run_bass_kernel_spmd

N_CORES = 8
M, C = 524288, 256
MLOC = M // N_CORES  # 65536 rows per core
P = 128  # SBUF partitions
R = 8  # rows per partition per tile
F = R * C  # free-dim elements per tile (2048)
T = MLOC // (P * R)  # tiles per core (64)
MM = 512  # moving free dim per matmul
F32 = mybir.dt.float32
XSTREAM = 6  # streaming/prefetch SBUF slots
NRES = 17  # tiles kept resident in SBUF between the passes


def build_nc():
    nc = bacc.Bacc("TRN2", target_bir_lowering=False, debug=False,
                   num_devices=N_CORES)
    x = nc.dram_tensor("x", [MLOC, C], F32, kind="ExternalInput")
    y = nc.dram_tensor("y", [MLOC, C], F32, kind="ExternalOutput")
    xt = x.ap().rearrange("(n p r) c -> n p (r c)", p=P, r=R)
    yt = y.ap().rearrange("(n p r) c -> n p (r c)", p=P, r=R)

    with tile.TileContext(nc) as tc:
        with (
            tc.tile_pool(name="xs", bufs=XSTREAM) as xs_pool,
            tc.tile_pool(name="xr", bufs=NRES) as xr_pool,
            tc.tile_pool(name="sqpool", bufs=2) as sqpool,
            tc.tile_pool(name="small", bufs=1) as spool,
            tc.tile_pool(name="psum", bufs=1, space="PSUM") as ppool,
            tc.tile_pool(name="dram", bufs=1, space="DRAM") as dpool,
        ):
            ones = spool.tile([P, 1], F32, tag="ones")
            nc.vector.memset(ones[:], 1.0)
            ps = ppool.tile([1, F], F32, tag="ps")
            # Warm the ACT sqrt table so the post-collective chain is short.
            warm = spool.tile([1, 4], F32, tag="warm")
            nc.vector.memset(warm[:], 1.0)
            nc.scalar.sqrt(warm[:], warm[:])

            # ---- pass 1: per-(row, column) sums of squares ----
            resident = {}
            for i in range(T):
                if i >= T - NRES:
                    xtile = xr_pool.tile([P, F], F32, tag="xr")
                    resident[i] = xtile
                else:
                    xtile = xs_pool.tile([P, F], F32, tag="xs")
                nc.sync.dma_start(xtile[:], xt[i])
                if i in resident:
                    # keep x intact for pass 2: square into scratch
                    sq = sqpool.tile([P, F], F32, tag="sq")
                else:
                    sq = xtile  # streamed tiles are re-read in pass 2
                nc.scalar.square(sq[:], xtile[:])
                for b in range(F // MM):
                    nc.tensor.matmul(
                        ps[:, b * MM:(b + 1) * MM],
                        ones[:],
                        sq[:, b * MM:(b + 1) * MM],
                        start=(i == 0),
                        stop=(i == T - 1),
                    )

            # ---- row-axis reduce + allreduce + rsqrt ----
            colsq = spool.tile([1, C], F32, tag="colsq")
            nc.vector.reduce_sum(
                colsq[:],
                ps[:].rearrange("p (r c) -> p c r", c=C),
                axis=mybir.AxisListType.X,
            )
            cin = dpool.tile([1, C], F32, tag="cin")
            cout = dpool.tile([1, C], F32, tag="cout")
            nc.gpsimd.dma_start(cin[:], colsq[:])
            nc.gpsimd.collective_compute(
                "AllReduce",
                mybir.AluOpType.add,
                replica_groups=[list(range(N_CORES))],
                ins=[cin.opt()],
                outs=[cout.opt()],
            )
            # Prefetch the first streamed pass-2 tiles on SWDGE; these use
            # separate completion lanes, so they cannot get FIFO-coupled
            # to the pass-2 stores.
            prefetched = {}
            for i in range(XSTREAM):
                pt = xs_pool.tile([P, F], F32, tag="xs")
                nc.gpsimd.dma_start(pt[:], xt[i])
                prefetched[i] = pt
            gsum = spool.tile([1, C], F32, tag="gsum")
            nc.scalar.dma_start(gsum[:], cout[:])
            inv = spool.tile([1, C], F32, tag="inv")
            nc.vector.reciprocal(inv[:], gsum[:])
            scl = spool.tile([1, C], F32, tag="scl")
            nc.scalar.sqrt(scl[:], inv[:])
            ones128 = spool.tile([1, P], F32, tag="ones128")
            nc.vector.memset(ones128[:], 1.0)
            sclb = ppool.tile([P, C], F32, tag="sclb")
            nc.tensor.matmul(sclb[:], ones128[:], scl[:], start=True, stop=True)

            # ---- pass 2: scale and write out ----
            # Resident tiles first (no load needed), then the prefetched
            # tiles, then re-stream the rest.
            sclb3 = sclb[:].unsqueeze(1).broadcast_to((P, R, C))
            n_stream = T - NRES
            order = (list(range(n_stream, T))
                     + list(range(XSTREAM))
                     + list(range(XSTREAM, n_stream)))
            for i in order:
                if i in resident:
                    xtile = resident[i]
                elif i in prefetched:
                    xtile = prefetched[i]
                else:
                    xtile = xs_pool.tile([P, F], F32, tag="xs")
                    nc.sync.dma_start(xtile[:], xt[i])
                v = xtile[:].rearrange("p (r c) -> p r c", c=C)
                nc.vector.tensor_mul(v, v, sclb3)
                nc.scalar.dma_start(yt[i], xtile[:])

    nc.compile()
    return nc


_NC_CACHE = None


def kernel(x) -> np.ndarray:
    global _NC_CACHE
    x = np.ascontiguousarray(np.asarray(x, dtype=np.float32))
    assert x.shape == (M, C)
    if _NC_CACHE is None:
        _NC_CACHE = build_nc()
    shards = x.reshape(N_CORES, MLOC, C)
    in_maps = [{"x": shards[i]} for i in range(N_CORES)]
    res = run_bass_kernel_spmd(_NC_CACHE, in_maps, list(range(N_CORES)))
    return np.concatenate([res.results[i]["y"] for i in range(N_CORES)], axis=0)


# revision 29
# speedup vs baseline: 1.1653x; 1.1653x over previous
"""Column-L2-normalization kernel for Trainium2 (8 NeuronCores, SPMD).

Computes y = x / sqrt(sum(x*x, axis=0)) for x of shape (524288, 256) fp32.

Strategy (row-sharded data parallel, single kernel launch):
  - Each of the 8 cores gets a contiguous shard of 65536 rows (64 MB).
  - Pass 1: stream 1 MB tiles ([128 partitions x 2048 fp32], 8 rows per
    partition), square on the scalar (ACT) engine, reduce over partitions
    with a ones-vector matmul accumulating into PSUM across all tiles.
    The last NRES tiles stay resident in SBUF (squared into scratch so
    the x data survives) and are not re-read in pass 2.
  - Reduce the row axis of the PSUM accumulator on DVE, AllReduce the
    256 per-column sums across the 8 cores, take 1/sqrt, and broadcast
    across partitions with a rank-1 matmul into PSUM.
  - Pass 2: resident tiles are scaled and stored immediately; the rest
    of the shard is re-streamed, scaled on DVE, and written out.
"""

import numpy as np

import concourse.bacc as bacc
import concourse.mybir as mybir
from concourse import tile
from concourse.bass_utils import run_bass_kernel_spmd

N_CORES = 8
M, C = 524288, 256
MLOC = M // N_CORES  # 65536 rows per core
P = 128  # SBUF partitions
R = 8  # rows per partition per tile
F = R * C  # free-dim elements per tile (2048)
T = MLOC // (P * R)  # tiles per core (64)
MM = 512  # moving free dim per matmul
F32 = mybir.dt.float32
XSTREAM = 6  # streaming/prefetch SBUF slots
NRES = 15  # tiles kept resident in SBUF between the passes


def build_nc():
    nc = bacc.Bacc("TRN2", target_bir_lowering=False, debug=False,
                   num_devices=N_CORES)
    x = nc.dram_tensor("x", [MLOC, C], F32, kind="ExternalInput")
    y = nc.dram_tensor("y", [MLOC, C], F32, kind="ExternalOutput")
    xt = x.ap().rearrange("(n p r) c -> n p (r c)", p=P, r=R)
    yt = y.ap().rearrange("(n p r) c -> n p (r c)", p=P, r=R)

    with tile.TileContext(nc) as tc:
        with (
            tc.tile_pool(name="xs", bufs=XSTREAM) as xs_pool,
            tc.tile_pool(name="xr", bufs=NRES) as xr_pool,
            tc.tile_pool(name="sqpool", bufs=2) as sqpool,
            tc.tile_pool(name="small", bufs=1) as spool,
            tc.tile_pool(name="psum", bufs=1, space="PSUM") as ppool,
            tc.tile_pool(name="dram", bufs=1, space="DRAM") as dpool,
        ):
            ones = spool.tile([P, 1], F32, tag="ones")
            nc.vector.memset(ones[:], 1.0)
            ps = ppool.tile([1, MM], F32, tag="ps")
            # Warm the ACT sqrt table so the post-collective chain is short.
            warm = spool.tile([1, 4], F32, tag="warm")
            nc.vector.memset(warm[:], 1.0)
            nc.scalar.sqrt(warm[:], warm[:])

            # ---- pass 1: per-(row, column) sums of squares ----
            resident = {}
            for i in range(T):
                if i >= T - NRES:
                    xtile = xr_pool.tile([P, F], F32, tag="xr")
                    resident[i] = xtile
                else:
                    xtile = xs_pool.tile([P, F], F32, tag="xs")
                nc.sync.dma_start(xtile[:], xt[i])
                if i in resident:
                    # keep x intact for pass 2: square into scratch
                    sq = sqpool.tile([P, F], F32, tag="sq")
                else:
                    sq = xtile  # streamed tiles are re-read in pass 2
                nc.scalar.square(sq[:], xtile[:])
                # fold twice on DVE so the PE streams only 512 fp32
                # columns per tile (fp32 rhs runs at 1/4 rate)
                h1 = sqpool.tile([P, F // 2], F32, tag="h1")
                nc.vector.tensor_add(h1[:], sq[:, :F // 2], sq[:, F // 2:])
                h2 = sqpool.tile([P, MM], F32, tag="h2")
                nc.vector.tensor_add(h2[:], h1[:, :MM], h1[:, MM:])
                nc.tensor.matmul(
                    ps[:], ones[:], h2[:],
                    start=(i == 0), stop=(i == T - 1),
                )

            # ---- row-axis reduce + allreduce + rsqrt ----
            colsq = spool.tile([1, C], F32, tag="colsq")
            nc.vector.reduce_sum(
                colsq[:],
                ps[:].rearrange("p (r c) -> p c r", c=C),
                axis=mybir.AxisListType.X,
            )
            cin = dpool.tile([1, C], F32, tag="cin")
            cout = dpool.tile([1, C], F32, tag="cout")
            nc.gpsimd.dma_start(cin[:], colsq[:])
            nc.gpsimd.collective_compute(
                "AllReduce",
                mybir.AluOpType.add,
                replica_groups=[list(range(N_CORES))],
                ins=[cin.opt()],
                outs=[cout.opt()],
            )
            # Prefetch the first streamed pass-2 tiles on SWDGE; these use
            # separate completion lanes, so they cannot get FIFO-coupled
            # to the pass-2 stores.
            prefetched = {}
            for i in range(XSTREAM):
                pt = xs_pool.tile([P, F], F32, tag="xs")
                nc.gpsimd.dma_start(pt[:], xt[i])
                prefetched[i] = pt
            gsum = spool.tile([1, C], F32, tag="gsum")
            nc.scalar.dma_start(gsum[:], cout[:])
            inv = spool.tile([1, C], F32, tag="inv")
            nc.vector.reciprocal(inv[:], gsum[:])
            scl = spool.tile([1, C], F32, tag="scl")
            nc.scalar.sqrt(scl[:], inv[:])
            ones128 = spool.tile([1, P], F32, tag="ones128")
            nc.vector.memset(ones128[:], 1.0)
            sclb = ppool.tile([P, C], F32, tag="sclb")
            nc.tensor.matmul(sclb[:], ones128[:], scl[:], start=True, stop=True)

            # ---- pass 2: scale and write out ----
            # Resident tiles first (no load needed), then the prefetched
            # tiles, then re-stream the rest.
            sclb3 = sclb[:].unsqueeze(1).broadcast_to((P, R, C))
            n_stream = T - NRES
            order = (list(range(n_stream, T))
                     + list(range(XSTREAM))
                     + list(range(XSTREAM, n_stream)))
            for i in order:
                if i in resident:
                    xtile = resident[i]
                elif i in prefetched:
                    xtile = prefetched[i]
                else:
                    xtile = xs_pool.tile([P, F], F32, tag="xs")
                    nc.sync.dma_start(xtile[:], xt[i])
                v = xtile[:].rearrange("p (r c) -> p r c", c=C)
                nc.vector.tensor_mul(v, v, sclb3)
                nc.scalar.dma_start(yt[i], xtile[:])

    nc.compile()
    return nc


_NC_CACHE = None


def kernel(x) -> np.ndarray:
    global _NC_CACHE
    x = np.ascontiguousarray(np.asarray(x, dtype=np.float32))
    assert x.shape == (M, C)
    if _NC_CACHE is None:
        _NC_CACHE = build_nc()
    shards = x.reshape(N_CORES, MLOC, C)
    in_maps = [{"x": shards[i]} for i in range(N_CORES)]
    res = run_bass_kernel_spmd(_NC_CACHE, in_maps, list(range(N_CORES)))
    return np.concatenate([res.results[i]["y"] for i in range(N_CORES)], axis=0)
